# revision 42
# baseline (speedup 1.0000x reference)
"""Trainium2 Bass kernel for nn_AugmentableSVSAlgorithm (scatter_memory).

Reference semantics:
  per-frame recurrence over T=2000 frames with carry (ht, lt) [128,160]:
      th = sigmoid((x - ht - d_hot) * hc);  tl = sigmoid((lt - x - d_hot) * hc)
      ht' = ht + d_open if x > ht else ht - d_close
      lt' = lt - d_open if lt > x else lt + d_close
      hot = th + tl
  then out = relu(1 - conv3x3(1 - hot, k, pad=1)).

Implementation (8 NeuronCores, SPMD, H-split sharding):
  * Core c owns rows [16c, 16c+16), processes rows [16c-1, 16c+17) (halo).
    Out-of-range halo rows and W-pad columns carry x = +1e6, which saturates
    both sigmoids so hot == 1.0 exactly — reproducing the reference conv's
    zero-padding of (1 - hot) with no edge cases anywhere.
  * State pairs S = [h | L], L = -lt: both thresholds share the update
    S' = select(x2 > S, S + d_open, S - d_close), x2 = [x | -x] (interleaved
    on the host). One fused custom VectorE op per time step — branch
    decisions and state arithmetic bit-exact vs the reference.
  * Layout per core: 126 partitions = (h:18, wb:7), pages of 25 cols
    (23-col stride + 2 halo cols) so the conv never crosses pages in the
    free dim. H direction on partitions via TensorE banded matrices.
  * VectorE runs ONLY the chain (~211ns/op hardware floor: one op per
    step, full pipeline drain between ops). The last op of each window
    deposits its carry directly into the next window's traj slot 0 (no
    copy, no boundary stall). Window sizes cascade small at both ends
    (6,14,30,50,...,40,30,14,10,6) so the first chain op starts after a
    6-step DMA and the post-chain tail is one tiny window's work; the
    first window's x2 DMA is hoisted ahead of the consts DMA.
  * E = hc*x2 - hc*S is produced by TensorE (scaled identity matmuls)
    into PSUM; ScalarE applies sigmoid straight from PSUM (fp16 out).
  * conv3x3 with the uniform kernel is separable: H-pass = 2 matmuls with
    an all-ones 3-tap h-band (exact in fp16, th + tl folded in by PSUM
    accumulation), ScalarE copies z to SBUF fp16, W-pass = 3 shifted
    identity matmuls; the kernel scale k00 is applied exactly in the
    ReLU activation's fp32 scale: out = relu((1-9*k00) + k00*zsum).
  * GpSimd does nothing — it shares SBUF ports with VectorE and measurably
    dilates every chain op (253 -> 313ns when it ran the E subtract).
"""

import numpy as np

T, H, W = 2000, 128, 160
NCORES = 8
ROWS = 18            # rows per core (16 own + 2 halo)
NB = 7               # W pages
PW = 25              # stored page width (23 real + 2 halo)
STRIDE = 23          # page stride in real-w
P = ROWS * NB        # 126 partitions
WIN = 100            # time steps per window
CE = 5               # steps per E/sigmoid chunk (50*CE <= 512 PSUM bank)
CH = 20              # steps per conv chunk (23*CH <= 512)
PAD = np.float32(1.0e6)

_OP = None


def _register_op():
    global _OP
    if _OP is not None:
        return _OP
    from concourse import dve_ops
    from concourse.dve_spec import Spec, Src0, Src1, C0, C1, select, lower
    from concourse.dve_uop import DveOpSpec

    name = "SVS_UPDATE_ANT"
    for o in dve_ops.OPS:
        if o.name == name:
            _OP = o
            return o
    spec = Spec(
        body=select(Src0 > Src1, Src1 + C0, Src1 - C1),
        reference=lambda in0, in1, c0, c1, c2: np.where(
            in0 > in1,
            (in1 + np.float32(c0)).astype(np.float32),
            (in1 - np.float32(c1)).astype(np.float32),
        ).astype(np.float32),
    )
    opcode = dve_ops._CUSTOM_DVE_ROW_BASE + len(dve_ops.OPS)
    shas = {}
    for ver in ("v3", "v4"):
        uops = lower(spec, ver=ver)
        shas[ver] = DveOpSpec(name=name, opcode=opcode, uops=uops, rd1_en=True).sha(ver)
    op = dve_ops.DveOp(name, spec, subdim=False, uops_sha=shas)
    dve_ops.OPS.append(op)
    dve_ops._SUB_OPCODE_FOR_NAME[name] = opcode
    dve_ops.CUSTOM_DVE_SPECS[name] = spec
    _OP = op
    return op


def _build_program(d_open, d_close, hc, hbias, k00, rbias):
    """One SPMD Bass program (same instruction stream on all 8 cores)."""
    from concourse import mybir, tile, bacc

    op = _register_op()
    nc = bacc.Bacc("TRN2", target_bir_lowering=False, debug=False,
                   num_devices=NCORES)
    f32 = mybir.dt.float32
    f16 = mybir.dt.float16
    xp_d = nc.dram_tensor("xp", [P, T, 2 * PW], f32, kind="ExternalInput").ap()
    s0_d = nc.dram_tensor("s0", [P, 2 * PW], f32, kind="ExternalInput").ap()
    eye_d = nc.dram_tensor("eye", [2, P, P], f32, kind="ExternalInput").ap()
    c16_d = nc.dram_tensor("c16", [2, P, P], f16, kind="ExternalInput").ap()
    out_d = nc.dram_tensor("out", [P, T, STRIDE], f32, kind="ExternalOutput").ap()

    Sig = mybir.ActivationFunctionType.Sigmoid
    Relu = mybir.ActivationFunctionType.Relu
    Copy = mybir.ActivationFunctionType.Copy
    FD = 2 * PW
    # cascaded first windows (the chain starts after only a 10-step DMA)
    # and last windows (the post-chain tail is one tiny window's work)
    if T == 2000:
        wins = [6, 14, 30, 50] + [WIN] * 18 + [40, 30, 14, 10, 6]
    else:
        wins = [WIN] * (T // WIN)
    assert sum(wins) == T

    with tile.TileContext(nc) as tc:
        with (
            tc.tile_pool(name="consts", bufs=1) as cpool,
            tc.tile_pool(name="x2", bufs=3) as x2pool,
            tc.tile_pool(name="traj", bufs=2) as tpool,
            tc.tile_pool(name="th", bufs=2) as thpool,
            tc.tile_pool(name="zz", bufs=2) as zpool,
            tc.tile_pool(name="outw", bufs=2) as opool,
            tc.tile_pool(name="eps", bufs=3, space="PSUM") as epspool,
            tc.tile_pool(name="zps", bufs=2, space="PSUM") as zpspool,
            tc.tile_pool(name="cps", bufs=3, space="PSUM") as cpspool,
        ):
            traj = tpool.tile([P, FD * (WIN + 1)], f32, tag="traj")
            x2_0 = x2pool.tile([P, FD * WIN], f32, tag="x2")
            nc.sync.dma_start(
                x2_0[:, 0:FD * 6].rearrange("p (t f) -> p t f", t=6),
                xp_d[:, 0:6, :],
            )
            nc.sync.dma_start(traj[:, 0:FD], s0_d[:])
            eyes = cpool.tile([P, 2 * P], f32)
            nc.sync.dma_start(eyes[:].rearrange("p (d q) -> p d q", d=2),
                              eye_d.rearrange("d p q -> p d q"))
            c16 = cpool.tile([P, 2 * P], f16)
            nc.sync.dma_start(c16[:].rearrange("p (d q) -> p d q", d=2),
                              c16_d.rearrange("d p q -> p d q"))
            eye16 = c16[:, 0:P]
            hband = c16[:, P:2 * P]
            hbias_t = cpool.tile([P, 1], f32)
            nc.vector.memset(hbias_t[:], hbias)
            rbias_t = cpool.tile([P, 1], f32)
            nc.vector.memset(rbias_t[:], rbias)

            t_base = 0
            for w, wlen in enumerate(wins):
                if w == 0:
                    x2 = x2_0
                else:
                    x2 = x2pool.tile([P, FD * WIN], f32, tag="x2")
                    nc.sync.dma_start(
                        x2[:, 0:FD * wlen].rearrange("p (t f) -> p t f", t=wlen),
                        xp_d[:, t_base:t_base + wlen, :],
                    )
                if w + 1 < len(wins):
                    traj_next = tpool.tile([P, FD * (WIN + 1)], f32, tag="traj",
                                           name=f"traj_next{w % 2}")
                else:
                    traj_next = None
                # the chain: one fused VectorE op per time step; the final
                # step deposits the carry straight into the next window
                for i in range(wlen):
                    dst = traj[:, FD * (i + 1):FD * (i + 2)]
                    if i == wlen - 1 and traj_next is not None:
                        dst = traj_next[:, 0:FD]
                    nc.vector._custom_dve(
                        op,
                        out=dst,
                        in0=x2[:, FD * i:FD * (i + 1)],
                        in1=traj[:, FD * i:FD * (i + 1)],
                        s0=d_open,
                        s1=d_close,
                    )
                # E = hc*x2 - hc*S_pre on TensorE; sigmoid from PSUM on ScalarE
                th = thpool.tile([P, FD * WIN], f16, tag="th")
                for c in range(0, wlen, CE):
                    ce = min(CE, wlen - c)
                    eps = epspool.tile([P, FD * CE], f32, tag="eps")
                    nc.tensor.matmul(
                        eps[:, 0:FD * ce], eyes[:, 0:P],
                        x2[:, FD * c:FD * (c + ce)], start=True, stop=False,
                    )
                    nc.tensor.matmul(
                        eps[:, 0:FD * ce], eyes[:, P:2 * P],
                        traj[:, FD * c:FD * (c + ce)], start=False, stop=True,
                    )
                    nc.scalar.activation(
                        th[:, FD * c:FD * (c + ce)], eps[:, 0:FD * ce], Sig,
                        bias=hbias_t[:], scale=1.0,
                    )
                # separable conv (uniform kernel): H-pass = 2 matmuls with an
                # all-ones 3-tap h-band (th + tl folded in by accumulation),
                # z to SBUF fp16, W-pass = 3 shifted identity matmuls; the
                # exact kernel scale k00 is applied in the ReLU's fp32 scale.
                outw = opool.tile([P, STRIDE * WIN], f32, tag="outw")
                z16 = zpool.tile([P, PW * WIN], f16, tag="z16")
                for cs in range(0, wlen, CH):
                    ch = min(CH, wlen - cs)
                    tchunk = th[:, FD * cs:FD * (cs + ch)].rearrange(
                        "p (t f) -> p t f", t=ch
                    )
                    zps = zpspool.tile([P, PW * CH], f32, tag="zps")
                    for j, half in enumerate((0, PW)):
                        nc.tensor.matmul(
                            zps[:, 0:PW * ch].rearrange("p (t f) -> p t f", t=ch),
                            hband, tchunk[:, :, half:half + PW],
                            start=(j == 0), stop=(j == 1),
                        )
                    nc.scalar.activation(
                        z16[:, PW * cs:PW * (cs + ch)], zps[:, 0:PW * ch],
                        Copy, bias=0.0, scale=1.0,
                    )
                    zchunk = z16[:, PW * cs:PW * (cs + ch)].rearrange(
                        "p (t f) -> p t f", t=ch
                    )
                    ps = cpspool.tile([P, STRIDE * CH], f32, tag="cps")
                    for kk, dw in enumerate((-1, 0, 1)):
                        nc.tensor.matmul(
                            ps[:, 0:STRIDE * ch].rearrange("p (t f) -> p t f", t=ch),
                            eye16, zchunk[:, :, 1 + dw:1 + dw + STRIDE],
                            start=(kk == 0), stop=(kk == 2),
                        )
                    nc.scalar.activation(
                        outw[:, STRIDE * cs:STRIDE * (cs + ch)],
                        ps[:, 0:STRIDE * ch], Relu, bias=rbias_t[:], scale=k00,
                    )
                nc.sync.dma_start(
                    out_d[:, t_base:t_base + wlen, :],
                    outw[:, 0:STRIDE * wlen].rearrange("p (t j) -> p t j", j=STRIDE),
                )
                t_base += wlen
                traj = traj_next
    nc.compile()
    return nc


_PROG_CACHE = {}


def _get_program(key, *args):
    if key not in _PROG_CACHE:
        _PROG_CACHE[key] = _build_program(*args)
    return _PROG_CACHE[key]


def _prep_inputs(x, params, ht0, lt0, kern, hc):
    """Build per-core input maps (host-side sharding)."""
    x = np.ascontiguousarray(x.reshape(T, H, W).astype(np.float32))
    ht0 = ht0.astype(np.float32)
    lt0 = lt0.astype(np.float32)
    kern = kern.astype(np.float32)

    # padded frame: rows [-1, H], cols [-1, W+2), pad value 1e6
    xp = np.full((T, H + 2, W + 3), PAD, np.float32)
    xp[:, 1:H + 1, 1:W + 1] = x
    hp = np.zeros((H + 2, W + 3), np.float32)
    hp[1:H + 1, 1:W + 1] = ht0
    lp = np.zeros((H + 2, W + 3), np.float32)
    lp[1:H + 1, 1:W + 1] = -lt0

    # E matrices: hc*I and -hc*I
    eye = np.zeros((2, P, P), np.float32)
    eye[0] = np.eye(P, dtype=np.float32) * np.float32(hc)
    eye[1] = np.eye(P, dtype=np.float32) * np.float32(-hc)
    # consts: +I fp16 and the all-ones 3-tap h-band fp16
    c16 = np.zeros((2, P, P), np.float16)
    c16[0] = np.eye(P, dtype=np.float16)
    for h_out in range(ROWS):
        for dy in (-1, 0, 1):
            h_in = h_out + dy
            if 0 <= h_in < ROWS:
                for wb in range(NB):
                    c16[1, h_in * NB + wb, h_out * NB + wb] = 1.0

    in_maps = []
    for c in range(NCORES):
        r0 = 16 * c
        xc = np.empty((ROWS, NB, T, 2 * PW), np.float32)
        sc = np.empty((ROWS, NB, 2 * PW), np.float32)
        for wb in range(NB):
            c0 = STRIDE * wb
            blk = xp[:, r0:r0 + ROWS, c0:c0 + PW].transpose(1, 0, 2)
            xc[:, wb, :, 0:PW] = blk
            xc[:, wb, :, PW:2 * PW] = -blk
            sc[:, wb, 0:PW] = hp[r0:r0 + ROWS, c0:c0 + PW]
            sc[:, wb, PW:2 * PW] = lp[r0:r0 + ROWS, c0:c0 + PW]
        in_maps.append({
            "xp": np.ascontiguousarray(xc.reshape(P, T, 2 * PW)),
            "s0": np.ascontiguousarray(sc.reshape(P, 2 * PW)),
            "eye": eye,
            "c16": c16,
        })
    return in_maps


TRACE = False        # test-harness hook: profile the SPMD run
LAST_RESULT = None


def kernel(x, params, ht0, lt0, kernel):
    global LAST_RESULT
    from concourse.bass_utils import run_bass_kernel_spmd

    p = np.asarray(params, np.float32)
    d_close, d_open, d_hot, hc = (float(p[0]), float(p[1]), float(p[2]), float(p[3]))
    kern = np.asarray(kernel, np.float32)
    assert float(np.abs(kern - kern[0, 0]).max()) == 0.0, "uniform kernel required"
    k00 = float(kern[0, 0])
    hbias = float(np.float32(-np.float32(d_hot) * np.float32(hc)))
    rbias = float(np.float32(1.0) - np.float32(9.0) * np.float32(k00))

    key = (d_close, d_open, d_hot, hc, kern.tobytes())
    nc = _get_program(key, d_open, d_close, hc, hbias, k00, rbias)
    in_maps = _prep_inputs(np.asarray(x), p, np.asarray(ht0), np.asarray(lt0),
                           kern, hc)
    r = run_bass_kernel_spmd(nc, in_maps, list(range(NCORES)), trace=TRACE)
    LAST_RESULT = r
    res = r.results
    out = np.empty((T, H, W), np.float32)
    for c in range(NCORES):
        out[:, 16 * c:16 * (c + 1), :] = _assemble(res[c]["out"])
    return out.reshape(T, 1, H, W).astype(np.float32)


def _assemble(raw):
    """[P, T, STRIDE] staging -> [T, 16, W] (drop halo rows h=0,17, pad cols)."""
    v = raw.reshape(ROWS, NB, T, STRIDE)[1:17]  # own rows
    full = v.transpose(2, 0, 1, 3).reshape(T, 16, NB * STRIDE)
    return full[:, :, :W]

